# revision 12
# baseline (speedup 1.0000x reference)
"""DeltaQGNN Trainium2 kernel (8 NeuronCores, sender-sharded).

Strategy: senders are sharded across the 8 cores in 8 aligned blocks of
SHARD=12544 node ids (host-side index-only preprocessing). Each core keeps
only the edges whose sender falls in its block, sorted by receiver. The
per-edge gather of sender q-rows runs through the SWDGE dma_gather custom
instruction (256B rows from a per-core DRAM table, int16 indices), chunked
through SBUF. Receiver segment sums use the chunk-reduce -> prefix-scan ->
boundary-difference scheme over a [128 x TCV] chunk grid per receiver
shard; boundary values are fetched with a second dma_gather from a DRAM
prefix-sum table. Per-core partial messages land in a shard-major buffer
that a ReduceScatter(add) collective turns into each core's own receiver
shard; the final affine combine (scalars folded on host) is node-sharded,
so each core returns [SHARD, F] and the host reassembles [F, N].
"""

from contextlib import ExitStack

import numpy as np
import ml_dtypes

import concourse.bass as bass
import concourse.tile as tile
from concourse import bacc, bass_utils, mybir

P = 128

# problem constants (hardcoded per contract)
N_FIELDS = 8
N_NODES = 100000
N_EDGES = 6400000
N_CORES = 8
F = N_FIELDS

# kernel configuration (real problem)
# NOTE: a single dma_gather may generate at most 128 SWDGE ring
# descriptors (num_idxs/16 + 1), so num_idxs per gather is capped at 1024.
REAL_CFG = dict(
    n_nodes=N_NODES,
    n_cores=N_CORES,
    sl=4,        # slots per chunk
    tcv=240,     # chunks per partition per shard (incl leading dummy)
    cg=1024,     # indices per gather chunk (SWDGE ring limit)
)

EL = 64          # f32 elements per gathered row (256B)


def _derive(cfg):
    n_nodes, n_cores = cfg["n_nodes"], cfg["n_cores"]
    sl, tcv, cg = cfg["sl"], cfg["tcv"], cfg["cg"]
    shard = ((n_nodes + n_cores * P - 1) // (n_cores * P)) * P
    d = dict(cfg)
    d.update(
        shard=shard,
        npad=shard * n_cores,
        nb=shard // P,
        lv=tcv * sl,
        sv=P * tcv * sl,
        qr=shard + P,       # gather table rows (dummy zero row at `shard`)
        dummy=shard,
    )
    assert d["sv"] % cg == 0
    assert P * tcv <= 32768, "boundary indices must fit int16"
    assert d["qr"] < 32768
    return d


def _prep(q, edges, senders, receivers, dt, w_self, w_msg, w_edge, b, cfg):
    """Host-side index preprocessing. Returns per-core in_maps."""
    c = _derive(cfg)
    n_cores, shard, npad = c["n_cores"], c["shard"], c["npad"]
    sl, tcv, lv, sv, nb = c["sl"], c["tcv"], c["lv"], c["sv"], c["nb"]
    dummy = c["dummy"]
    n_nodes = c["n_nodes"]

    x = np.ascontiguousarray(edges[:, 0]).astype(np.float32)
    senders = senders.astype(np.int64)
    receivers = receivers.astype(np.int64)

    # scalars folded with dt (as in the affine combine)
    dtv = np.float32(dt[0])
    scal = np.zeros((P, 32), np.float32)
    scal[:, 0:8] = (dtv * w_self).astype(np.float32)
    scal[:, 8:16] = (dtv * w_msg).astype(np.float32)
    scal[:, 16:24] = (dtv * w_msg * w_edge).astype(np.float32)
    scal[:, 24:32] = (dtv * b).astype(np.float32)

    # one global sort: (sender-core, receiver)
    core_of = senders // shard
    perm = np.argsort(core_of * np.int64(npad) + receivers, kind="stable")
    s_s = senders[perm]
    r_s = receivers[perm]
    x_s = x[perm]
    core_starts = np.searchsorted(core_of[perm], np.arange(n_cores + 1))

    in_maps = []
    for core in range(n_cores):
        i0, i1 = int(core_starts[core]), int(core_starts[core + 1])
        sl_loc = (s_s[i0:i1] - core * shard).astype(np.int64)
        r_loc = r_s[i0:i1]
        x_loc = x_s[i0:i1]

        cnt = np.bincount(r_loc, minlength=npad).astype(np.int64)
        chm_all = (cnt + sl - 1) // sl

        idxg = np.full((n_cores, P, lv), dummy, np.int16)
        xsg = np.zeros((n_cores, P, lv), np.float32)
        bs_all = np.zeros(npad, np.int64)
        be_all = np.zeros(npad, np.int64)

        for m in range(n_cores):
            nlo, nhi = m * shard, (m + 1) * shard
            chm = chm_all[nlo:nhi]
            T = int(chm.sum())
            assert T <= P * (tcv - 1), (core, m, T)
            cum = np.cumsum(chm)
            cuts = np.ceil(T * np.arange(1, P) / P).astype(np.int64)
            bounds = np.concatenate(
                [[0], np.searchsorted(cum, cuts, side="left") + 1, [shard]])
            bounds = np.minimum(bounds, shard)
            bounds = np.maximum.accumulate(bounds)
            npp = np.diff(bounds)
            pa = np.repeat(np.arange(P), npp)
            cstart = cum - chm
            pstart = np.append(cstart, T)[bounds[:-1]]
            c0 = cstart - pstart[pa] + 1          # first chunk (after dummy)
            c1 = c0 + chm
            assert int(c1.max(initial=0)) <= tcv, (core, m)
            bs_all[nlo:nhi] = pa * tcv + c0 - 1
            be_all[nlo:nhi] = pa * tcv + c1 - 1

            # scatter this shard's edges into the slot grid
            j0 = int(np.searchsorted(r_loc, nlo))
            j1 = int(np.searchsorted(r_loc, nhi))
            rr = r_loc[j0:j1] - nlo
            cumcnt = np.cumsum(cnt[nlo:nhi])
            rank = np.arange(j1 - j0) - (cumcnt - cnt[nlo:nhi])[rr]
            slot = c0[rr] * sl + rank
            assert int(slot.max(initial=0)) < lv
            idxg[m, pa[rr], slot] = sl_loc[j0:j1].astype(np.int16)
            xsg[m, pa[rr], slot] = x_loc[j0:j1]

        # wrapped idx layout: stream j = l*128 + p ; wrapped (j%16, j//16)
        idx_stream = idxg.transpose(0, 2, 1).reshape(n_cores, sv)
        idxw = (idx_stream.reshape(n_cores, sv // 16, 16)
                .transpose(2, 0, 1).reshape(16, n_cores * (sv // 16)))
        idxw = np.ascontiguousarray(idxw)

        xsw = np.ascontiguousarray(
            xsg.transpose(1, 0, 2).reshape(P, n_cores * lv)
        ).astype(ml_dtypes.float8_e4m3)

        # boundary streams, per shard: [bs block | be block], wrapped
        bw = np.zeros((16, n_cores * 2 * (shard // 16)), np.int16)
        hw = shard // 16
        for m in range(n_cores):
            bs_m = bs_all[m * shard:(m + 1) * shard]
            be_m = be_all[m * shard:(m + 1) * shard]
            off = m * 2 * hw
            bw[:, off:off + hw] = bs_m.reshape(hw, 16).T
            bw[:, off + hw:off + 2 * hw] = be_m.reshape(hw, 16).T

        qc = np.zeros((shard, F), np.float32)
        lo = core * shard
        hi = min(lo + shard, n_nodes)
        qc[:hi - lo] = q[:, lo:hi].T

        in_maps.append({
            "qc": qc,
            "idxw": idxw,
            "xsw": xsw,
            "bw": bw,
            "scal": scal,
        })
    return in_maps


def _build_nc(cfg):
    c = _derive(cfg)
    n_cores, shard = c["n_cores"], c["shard"]
    sl, tcv, cg, lv, sv, nb, qr = (c["sl"], c["tcv"], c["cg"], c["lv"],
                                   c["sv"], c["nb"], c["qr"])
    f32, i16, f8 = mybir.dt.float32, mybir.dt.int16, mybir.dt.float8e4
    G = F + 1

    nc = bacc.Bacc("TRN2", target_bir_lowering=False, debug=False,
                   num_devices=n_cores)
    qc = nc.dram_tensor("qc", [shard, F], f32, kind="ExternalInput")
    idxw = nc.dram_tensor("idxw", [16, n_cores * (sv // 16)], i16,
                          kind="ExternalInput")
    xsw = nc.dram_tensor("xsw", [P, n_cores * lv], f8, kind="ExternalInput")
    bw = nc.dram_tensor("bw", [16, n_cores * 2 * (shard // 16)], i16,
                        kind="ExternalInput")
    scal = nc.dram_tensor("scal", [P, 32], f32, kind="ExternalInput")

    qtab = nc.dram_tensor("qtab", [qr, EL], f32, kind="Internal")
    s2tabs = [nc.dram_tensor(f"s2tab{m}", [P * tcv, EL], f32, kind="Internal")
              for m in range(n_cores)]
    pmsg = nc.dram_tensor("pmsg", [n_cores * shard, G], f32, kind="Internal")
    rmsg = nc.dram_tensor("rmsg", [shard, G], f32, kind="Internal")
    out = nc.dram_tensor("out", [shard, F], f32, kind="ExternalOutput")

    with tile.TileContext(nc) as tc, ExitStack() as ctx:
        io = ctx.enter_context(tc.tile_pool(name="io", bufs=2))
        bnd = ctx.enter_context(tc.tile_pool(name="bnd", bufs=1))
        acc = ctx.enter_context(tc.tile_pool(name="acc", bufs=1))

        # zero-fill gather tables (dummy rows / unused columns)
        zw = max((qr // P) * EL, (tcv // 4) * EL)
        z = acc.tile([P, zw], f32)
        nc.vector.memset(z[:], 0.0)
        nc.sync.dma_start(
            qtab.ap().rearrange("(t p) e -> p t e", p=P),
            z[:, :(qr // P) * EL].rearrange("p (t e) -> p t e", e=EL))

        # q shard -> SBUF grid and gather table rows [0:shard), cols 0:F
        qs = acc.tile([P, nb * F], f32)
        nc.sync.dma_start(
            qs[:].rearrange("p (t f) -> p t f", f=F),
            qc.ap().rearrange("(t p) f -> p t f", p=P))
        nc.sync.dma_start(
            qtab.ap()[0:shard, :].rearrange("(t p) e -> p t e", p=P)[:, :, 0:F],
            qs[:].rearrange("p (t f) -> p t f", f=F))

        scal_t = acc.tile([P, 32], f32)
        nc.sync.dma_start(scal_t[:], scal.ap()[:])

        for m in range(n_cores):
            iw = sv // 16
            idxsb = io.tile([P, iw], i16, tag="idx")
            for g in range(8):
                nc.sync.dma_start(idxsb[16 * g:16 * (g + 1), :],
                                  idxw.ap()[:, m * iw:(m + 1) * iw])
            xst = io.tile([P, lv], f8, tag="xs")
            nc.sync.dma_start(xst[:], xsw.ap()[:, m * lv:(m + 1) * lv])

            L2 = io.tile([P, tcv * F], f32, tag="L2")
            xL2 = io.tile([P, tcv], f32, tag="xL2")
            nc.vector.tensor_reduce(
                out=xL2[:],
                in_=xst[:].rearrange("p (t s) -> p t s", s=sl),
                axis=mybir.AxisListType.X, op=mybir.AluOpType.add)

            tk = cg // P // sl   # chunks per gather block
            for k in range(sv // cg):
                v = io.tile([P, (cg // P) * EL], f32, tag="v")
                nc.gpsimd.dma_gather(
                    out_ap=v[:].rearrange("p (t e) -> p t e", e=EL),
                    in_ap=qtab.ap()[:],
                    idxs_ap=idxsb[:, k * (cg // 16):(k + 1) * (cg // 16)],
                    num_idxs=cg, num_idxs_reg=cg, elem_size=EL)
                vv = v[:].rearrange("p (t s e) -> p t e s", s=sl, e=EL)
                nc.vector.tensor_reduce(
                    out=L2[:].rearrange("p (t f) -> p t f", f=F)
                    [:, k * tk:(k + 1) * tk, :],
                    in_=vv[:, :, 0:F, :],
                    axis=mybir.AxisListType.X, op=mybir.AluOpType.add)

            S2 = io.tile([P, tcv * F], f32, tag="S2")
            xS2 = io.tile([P, tcv], f32, tag="xS2")
            L2v = L2[:].rearrange("p (t f) -> p f t", f=F)
            S2v = S2[:].rearrange("p (t f) -> p f t", f=F)
            for f in range(F):
                nc.vector.tensor_tensor_scan(
                    out=S2v[:, f, :], data0=L2v[:, f, :], data1=L2v[:, f, :],
                    initial=0.0, op0=mybir.AluOpType.add,
                    op1=mybir.AluOpType.bypass)
            nc.vector.tensor_tensor_scan(
                out=xS2[:], data0=xL2[:], data1=xL2[:],
                initial=0.0, op0=mybir.AluOpType.add,
                op1=mybir.AluOpType.bypass)

            s2v = s2tabs[m].ap().rearrange("(p t) e -> p t e", p=P)
            nc.sync.dma_start(s2v[:, :, 0:F],
                              S2[:].rearrange("p (t f) -> p t f", f=F))
            nc.sync.dma_start(s2v[:, :, F:F + 1], xS2[:].unsqueeze(2))

            # boundary lookups: [bs block | be block] in one gather
            bwc = 2 * (shard // 16)
            bidx = bnd.tile([P, bwc], i16, tag="bidx")
            for g in range(8):
                nc.sync.dma_start(bidx[16 * g:16 * (g + 1), :],
                                  bw.ap()[:, m * bwc:(m + 1) * bwc])
            eb = bnd.tile([P, 2 * nb * EL], f32, tag="eb")
            ebg = eb[:].rearrange("p (t e) -> p t e", e=EL)
            for j0 in range(0, 2 * shard, cg):
                nj = min(cg, 2 * shard - j0)
                nc.gpsimd.dma_gather(
                    out_ap=ebg[:, j0 // P:(j0 + nj) // P, :],
                    in_ap=s2tabs[m].ap()[:],
                    idxs_ap=bidx[:, j0 // 16:(j0 + nj) // 16],
                    num_idxs=nj, num_idxs_reg=nj, elem_size=EL)
            diff = bnd.tile([P, nb * G], f32, tag="diff")
            ebv = eb[:].rearrange("p (t e) -> p t e", e=EL)
            nc.vector.tensor_tensor(
                out=diff[:].rearrange("p (t g) -> p t g", g=G),
                in0=ebv[:, nb:2 * nb, 0:G], in1=ebv[:, 0:nb, 0:G],
                op=mybir.AluOpType.subtract)
            nc.sync.dma_start(
                pmsg.ap()[m * shard:(m + 1) * shard, :]
                .rearrange("(t p) g -> p t g", p=P),
                diff[:].rearrange("p (t g) -> p t g", g=G))

        nc.gpsimd.collective_compute(
            "ReduceScatter",
            mybir.AluOpType.add,
            replica_groups=[list(range(n_cores))],
            ins=[pmsg.ap()[:].opt()],
            outs=[rmsg.ap()[:].opt()],
        )

        msgc = acc.tile([P, nb * G], f32)
        nc.sync.dma_start(
            msgc[:].rearrange("p (t g) -> p t g", g=G),
            rmsg.ap().rearrange("(t p) g -> p t g", p=P))

        qvv = qs[:].rearrange("p (t f) -> p t f", f=F)
        mv = msgc[:].rearrange("p (t g) -> p t g", g=G)
        A = scal_t[:, 0:8].unsqueeze(1).to_broadcast([P, nb, F])
        B = scal_t[:, 8:16].unsqueeze(1).to_broadcast([P, nb, F])
        Cx = scal_t[:, 16:24].unsqueeze(1).to_broadcast([P, nb, F])
        D = scal_t[:, 24:32].unsqueeze(1).to_broadcast([P, nb, F])
        o1 = acc.tile([P, nb * F], f32)
        o2 = acc.tile([P, nb * F], f32)
        o1v = o1[:].rearrange("p (t f) -> p t f", f=F)
        o2v = o2[:].rearrange("p (t f) -> p t f", f=F)
        nc.vector.tensor_tensor(out=o1v, in0=qvv, in1=A, op=mybir.AluOpType.mult)
        nc.vector.tensor_tensor(out=o2v, in0=mv[:, :, 0:F], in1=B,
                                op=mybir.AluOpType.mult)
        nc.vector.tensor_tensor(out=o1v, in0=o1v, in1=o2v, op=mybir.AluOpType.add)
        nc.vector.tensor_tensor(out=o2v,
                                in0=mv[:, :, F:F + 1].to_broadcast([P, nb, F]),
                                in1=Cx, op=mybir.AluOpType.mult)
        nc.vector.tensor_tensor(out=o1v, in0=o1v, in1=o2v, op=mybir.AluOpType.add)
        nc.vector.tensor_tensor(out=o1v, in0=o1v, in1=D, op=mybir.AluOpType.add)
        nc.sync.dma_start(out.ap().rearrange("(t p) f -> p t f", p=P), o1v)

    nc.compile()
    return nc


_NC_CACHE = {}
_EXEC_CACHE = {}


def _build_nc_cached(cfg):
    key = tuple(sorted(cfg.items()))
    if key not in _NC_CACHE:
        _NC_CACHE[key] = _build_nc(cfg)
    return _NC_CACHE[key]


def _make_runner(nc):
    """Compile-once runner mirroring bass2jax.run_bass_via_pjrt's multi-core
    path, so repeated invocations skip jit re-tracing / executable rebuild."""
    import jax
    from jax.sharding import Mesh, PartitionSpec
    from jax.experimental.shard_map import shard_map
    from concourse import bass2jax

    bass2jax.install_neuronx_cc_hook()
    n_cores = nc.num_devices
    partition_name = (nc.partition_id_tensor.name
                      if nc.partition_id_tensor else None)
    in_names, out_names, out_avals = [], [], []
    for alloc in nc.m.functions[0].allocations:
        if not isinstance(alloc, mybir.MemoryLocationSet):
            continue
        name = alloc.memorylocations[0].name
        if alloc.kind == "ExternalInput":
            if name != partition_name:
                in_names.append(name)
        elif alloc.kind == "ExternalOutput":
            out_names.append(name)
            out_avals.append(jax.core.ShapedArray(
                tuple(alloc.tensor_shape), mybir.dt.np(alloc.dtype)))
    n_params = len(in_names)
    n_outs = len(out_avals)
    all_names = in_names + out_names
    if partition_name is not None:
        all_names.append(partition_name)
    donate = tuple(range(n_params, n_params + n_outs))

    def _body(*args):
        operands = list(args)
        if partition_name is not None:
            operands.append(bass2jax.partition_id_tensor())
        outs = bass2jax._bass_exec_p.bind(
            *operands, out_avals=tuple(out_avals), in_names=tuple(all_names),
            out_names=tuple(out_names), lowering_input_output_aliases=(),
            sim_require_finite=True, sim_require_nnan=True, nc=nc)
        return tuple(outs)

    devices = jax.devices()[:n_cores]
    mesh = Mesh(np.asarray(devices), ("core",))
    fn = jax.jit(
        shard_map(_body, mesh=mesh,
                  in_specs=(PartitionSpec("core"),) * (n_params + n_outs),
                  out_specs=(PartitionSpec("core"),) * n_outs,
                  check_rep=False),
        donate_argnums=donate, keep_unused=True)

    def run(in_maps):
        concat_in = [
            np.concatenate([np.asarray(m[nm]) for m in in_maps], axis=0)
            for nm in in_names]
        concat_zeros = [
            np.zeros((n_cores * a.shape[0], *a.shape[1:]), a.dtype)
            for a in out_avals]
        outs = fn(*concat_in, *concat_zeros)
        return [
            {name: np.asarray(outs[i]).reshape(n_cores, *out_avals[i].shape)[c]
             for i, name in enumerate(out_names)}
            for c in range(n_cores)]

    return run


def _run(nc, in_maps):
    key = id(nc)
    if key not in _EXEC_CACHE:
        _EXEC_CACHE[key] = _make_runner(nc)
    return _EXEC_CACHE[key](in_maps)


def kernel(q, edges, senders, receivers, dt, w_self, w_msg, w_edge, b):
    q = np.asarray(q, dtype=np.float32)
    edges = np.asarray(edges, dtype=np.float32)
    senders = np.asarray(senders, dtype=np.int32)
    receivers = np.asarray(receivers, dtype=np.int32)
    dt = np.asarray(dt, dtype=np.float32)
    w_self = np.asarray(w_self, dtype=np.float32)
    w_msg = np.asarray(w_msg, dtype=np.float32)
    w_edge = np.asarray(w_edge, dtype=np.float32)
    b = np.asarray(b, dtype=np.float32)

    in_maps = _prep(q, edges, senders, receivers, dt, w_self, w_msg, w_edge, b,
                    REAL_CFG)
    nc = _build_nc_cached(REAL_CFG)
    results = _run(nc, in_maps)

    c = _derive(REAL_CFG)
    shard = c["shard"]
    full = np.empty((F, N_NODES), dtype=np.float32)
    for core in range(N_CORES):
        o = results[core]["out"]
        lo = core * shard
        hi = min(lo + shard, N_NODES)
        full[:, lo:hi] = o[:hi - lo].T
    return full
